# revision 38
# baseline (speedup 1.0000x reference)
"""Trainium2 Bass kernel for Llama-style GQA attention prefill (S=2048).

Sharding: tensor-parallel over heads across 8 NeuronCores.
Each core owns 4 query heads + 1 KV head (GQA group-aligned), computes
its partial o_proj contribution (Wo input-sharded), and the host sums
the 8 partials.

Math notes:
- The paged-KV write+gather in the reference is an identity whenever
  page_indices are distinct (they are: arange(128)), so the kernel
  computes plain causal GQA attention with RoPE.
- Matmuls run in bf16 (4x faster than fp32 on the PE) with fp32 PSUM
  accumulation. Score scale 1/sqrt(D) is folded into Wq on the host.
- Attention uses a transposed-score layout: scoresT[k, q] so softmax
  needs no PE transposes. exp() is taken without max subtraction
  (scores are O(10), safe in fp32).

Performance structure (per core):
- Phase 1 (projections): W stationary / x moving, N=512 tiles; RoPE's
  half-swap done with a bf16 PE permutation matmul (4x faster than the
  fp32 variant; DVE stream_shuffle cannot cross 32-partition quadrants);
  w/x DMAs interleaved so the first matmul starts ~1us in.
- Phase 2 (attention): kt-major over head pairs with a one-pair
  software pipeline so the PE never waits on the scalar-engine exp;
  the 4 heads' softmax denominators accumulate into rows 0/32/64/96 of
  a single PSUM bank (shifted one-hot stationary operands), inverted
  with reciprocal_approx_fast and broadcast via a bf16 rank-1 matmul.
  All of this stays off the PE critical path, keeping the HAM clock
  gate warm (the previous version lost ~80us to K=4/8 throttling).
- Phase 3 (o_proj): at[h] stationary shared across 4 hidden blocks
  (psum-bank-grouped), bf16 output (halves the output DMA).
"""

import sys

if "/opt/trn_rl_repo" not in sys.path:
    sys.path.insert(0, "/opt/trn_rl_repo")

import numpy as np
import ml_dtypes

BF = ml_dtypes.bfloat16

S = 2048
HID = 4096
D = 128
H = 32
HKV = 8
NCORES = 8
NQ = H // NCORES  # 4 query heads per core
ROPE_THETA = 10000.0

_NC_CACHE = {}


def build_nc(s=S, hid=HID, nq=NQ):
    """Build the per-core Bass program (same program for all 8 cores)."""
    import concourse.bass as bass
    import concourse.mybir as mybir
    import concourse.tile as tile
    from concourse import bacc
    from concourse.masks import make_identity

    f32 = mybir.dt.float32
    bf16 = mybir.dt.bfloat16
    Exp = mybir.ActivationFunctionType.Exp

    KB = hid // 128   # hidden contraction blocks
    SBn = s // 512    # 512-wide sequence blocks
    KTn = s // 128    # 128-wide key tiles
    STn = s // 128    # 128-wide seq tiles (same count)
    NDB = nq + 2      # projection d-blocks: k, v, q0..q{nq-1}
    WC = NDB * 128    # wqkvT columns
    HB = hid // 512   # output hidden blocks

    nc = bacc.Bacc("TRN2")

    xT_d = nc.dram_tensor("xT", [hid, s], bf16, kind="ExternalInput")
    w_d = nc.dram_tensor("wqkvT", [hid, WC], bf16, kind="ExternalInput")
    wo_d = nc.dram_tensor("woT", [nq * 128, hid], bf16, kind="ExternalInput")
    cos2_d = nc.dram_tensor("cos2", [128, s], bf16, kind="ExternalInput")
    sin2_d = nc.dram_tensor("sin2", [128, s], bf16, kind="ExternalInput")
    tri_d = nc.dram_tensor("tri", [128, 128], bf16, kind="ExternalInput")
    swpm_d = nc.dram_tensor("swpm", [128, 128], bf16, kind="ExternalInput")
    out_d = nc.dram_tensor("out", [s, hid], bf16, kind="ExternalOutput")

    with tile.TileContext(nc) as tc:
        with (
            tc.tile_pool(name="const", bufs=1) as const_pool,
            tc.tile_pool(name="qkv", bufs=1) as qkv_pool,
        ):
            tri = const_pool.tile([128, 128], bf16, tag="tri")
            # per-head one-hot slabs: oh4[:, h, 32h] = 1, zero elsewhere
            oh4 = const_pool.tile([128, nq, 128], bf16, tag="oh4")
            ones32 = const_pool.tile([128, 128], bf16, tag="ones32")
            ident = const_pool.tile([128, 128], bf16, tag="ident")
            swpm = const_pool.tile([128, 128], bf16, tag="swpm")
            nc.sync.dma_start(tri, tri_d[:, :])
            nc.sync.dma_start(swpm, swpm_d[:, :])
            nc.vector.memset(oh4, 0.0)
            for h in range(nq):
                nc.vector.memset(oh4[:, h, 32 * h : 32 * h + 1], 1.0)
            nc.vector.memset(ones32, 1.0)
            make_identity(nc, ident)

            # persistent per-head tensors
            qk = [
                qkv_pool.tile([128, s], bf16, tag=f"qk{i}", name=f"qk{i}")
                for i in range(nq + 1)
            ]  # qk[0..nq-1] = q heads (T layout [d, s]); qk[nq] = kT
            vt = qkv_pool.tile([128, KTn, 128], bf16, tag="vt")  # v natural [s,d] tiles

            # ---------------- Phase 1: projections + RoPE ----------------
            with (
                tc.tile_pool(name="cs", bufs=1) as cs_pool,
                tc.tile_pool(name="xt", bufs=2) as xt_pool,
                tc.tile_pool(name="wsb", bufs=1) as w_pool,
                tc.tile_pool(name="pp", bufs=4, space="PSUM") as pp,
                tc.tile_pool(name="tpp", bufs=2, space="PSUM") as tpp,
                tc.tile_pool(name="spp", bufs=2, space="PSUM") as spp,
                tc.tile_pool(name="rtmp", bufs=2) as rt,
                tc.tile_pool(name="vstage", bufs=2) as vs,
            ):
                cos2 = cs_pool.tile([128, s], bf16, tag="cos2")
                sin2 = cs_pool.tile([128, s], bf16, tag="sin2")
                w_sb = w_pool.tile([128, KB, WC], bf16, tag="wsb")
                wv_view = w_d[:, :].rearrange("(t p) c -> p t c", p=128)

                # interleave w and first-seq-block x DMAs so the first
                # matmul can start after ~2 transfers instead of ~6 MB
                xts_all = []
                for sb in range(SBn):
                    xts_all.append(
                        [
                            xt_pool.tile(
                                [128, 512], bf16, tag=f"xt{kb}", name=f"xt{kb}"
                            )
                            for kb in range(KB)
                        ]
                    )
                for kb in range(KB):
                    nc.sync.dma_start(w_sb[:, kb, :], wv_view[:, kb, :])
                    nc.sync.dma_start(
                        xts_all[0][kb], xT_d[kb * 128 : (kb + 1) * 128, 0:512]
                    )
                    if kb == 24:
                        # cos/sin aren't needed until the first RoPE block
                        # (~40us in); issuing them here keeps the first
                        # matmuls' w/x transfers at the head of the queue
                        nc.sync.dma_start(cos2, cos2_d[:, :])
                        nc.sync.dma_start(sin2, sin2_d[:, :])

                # d-block order: k(0), v(1), then q heads (2..)
                for sb in range(SBn):
                    sl = slice(sb * 512, (sb + 1) * 512)
                    xts = xts_all[sb]
                    if sb > 0:
                        for kb in range(KB):
                            nc.sync.dma_start(
                                xts[kb], xT_d[kb * 128 : (kb + 1) * 128, sl]
                            )
                    for db in range(NDB):
                        ps = pp.tile([128, 512], f32, tag="pp")
                        for kb in range(KB):
                            nc.tensor.matmul(
                                ps,
                                w_sb[:, kb, db * 128 : (db + 1) * 128],
                                xts[kb],
                                start=(kb == 0),
                                stop=(kb == KB - 1),
                            )
                        if db == 1:
                            # v: cast to bf16 then transpose to natural [s, d]
                            vstg = vs.tile([128, 512], bf16, tag="vstg")
                            nc.scalar.copy(vstg, ps)
                            for j in range(4):
                                tps = tpp.tile([128, 128], bf16, tag="tpp")
                                nc.tensor.transpose(
                                    tps, vstg[:, j * 128 : (j + 1) * 128], ident
                                )
                                nc.vector.tensor_copy(vt[:, sb * 4 + j, :], tps)
                        else:
                            # RoPE: dst = p * COS2 + halfswap(p) * SIN2
                            # (half-swap via a bf16 PE permutation matmul;
                            # sin2 rows 0-63 hold -sin so one form works)
                            dst = qk[nq] if db == 0 else qk[db - 2]
                            pcs = rt.tile([128, 512], bf16, tag="pcs")
                            nc.scalar.copy(pcs, ps)
                            sps = spp.tile([128, 512], f32, tag="sps")
                            nc.tensor.matmul(sps, swpm, pcs, start=True, stop=True)
                            m1 = rt.tile([128, 512], f32, tag="m1")
                            nc.vector.tensor_mul(m1, ps, cos2[:, sl])
                            swp = rt.tile([128, 512], f32, tag="swp")
                            nc.vector.tensor_mul(swp, sps, sin2[:, sl])
                            nc.vector.tensor_add(dst[:, sl], m1, swp)

            # ---------------- Phase 2: attention ----------------
            kT = qk[nq]
            with (
                tc.tile_pool(name="wosb", bufs=1) as wo_pool,
                # dsm outlives phase 2: the last block's normalization is
                # emitted inside phase 3 (its bcast matmuls read dnrb there)
                tc.tile_pool(name="dsm", bufs=2) as dsm,
            ):
                at = [
                    wo_pool.tile([128, s], bf16, tag=f"at{i}", name=f"at{i}")
                    for i in range(nq)
                ]  # attnT per head [d, s]
                wo_sb = wo_pool.tile([128, nq, hid], bf16, tag="wosb")
                nc.sync.dma_start(
                    wo_sb, wo_d[:, :].rearrange("(t p) c -> p t c", p=128)
                )
                with (
                    # 3 score banks + 1 denominator bank (was 2+2): the
                    # extra score slack keeps the PE fed across block
                    # boundaries; the dn rotation dependency is covered
                    # by the reciprocal finishing ~1us into each block
                    tc.tile_pool(name="scp", bufs=3, space="PSUM") as scp,
                    tc.tile_pool(name="atp", bufs=1, space="PSUM") as atp,
                    tc.tile_pool(name="dnp", bufs=1, space="PSUM") as dnp,
                    tc.tile_pool(name="exps", bufs=3) as exps,
                ):
                    pending_norm = None

                    for qb in range(SBn):
                        nkt = 4 * qb + 4
                        accs = [
                            atp.tile(
                                [128, 512], f32, tag=f"acc{h}", name=f"acc{h}"
                            )
                            for h in range(nq)
                        ]
                        dnB = dnp.tile([128, 512], f32, tag="dn")

                        # software pipeline over (kt, head-pair) with one
                        # pair of lag so dn/acc matmuls never wait on exp
                        pending_pair = None
                        pair_idx = 0
                        for kt in range(nkt):
                            jstart = max(0, 128 * (kt - 4 * qb))
                            w = 512 - jstart
                            q_lo = qb * 512 + jstart
                            diag = kt >= 4 * qb
                            for hp in range(2):
                                exs = []
                                for h in (2 * hp, 2 * hp + 1):
                                    sc = scp.tile([128, 512], f32, tag="sc")
                                    nc.tensor.matmul(
                                        sc[:, :w],
                                        kT[:, kt * 128 : (kt + 1) * 128],
                                        qk[h][:, q_lo : (qb + 1) * 512],
                                        start=True,
                                        stop=True,
                                    )
                                    ex = exps.tile(
                                        [128, 512], bf16, tag=f"ex{h}",
                                        name=f"ex{h}",
                                    )
                                    nc.scalar.activation(
                                        ex[:, :w], sc[:, :w], Exp
                                    )
                                    if diag:
                                        nc.vector.tensor_mul(
                                            ex[:, 0:128], ex[:, 0:128], tri
                                        )
                                    exs.append((h, ex))
                                if pending_pair is not None:
                                    _emit_dn_acc(
                                        nc, pending_pair, oh4, vt, dnB, accs,
                                        qb, nkt,
                                    )
                                pending_pair = (kt, jstart, w, exs)
                                pair_idx += 1
                                if pair_idx == 2 and pending_norm is not None:
                                    # previous block's normalizing broadcast:
                                    # two pairs in, the reciprocal chain has
                                    # certainly finished, so the PE won't wait
                                    pending_norm(
                                        lambda h: scp.tile(
                                            [128, 512], f32, tag="sc",
                                            name="bc",
                                        )
                                    )
                                    pending_norm = None
                        _emit_dn_acc(
                            nc, pending_pair, oh4, vt, dnB, accs, qb, nkt
                        )

                        # epilogue: invert denominators (rows 0/32/64/96 of
                        # dnB) and copy unnormalized acc into at[h]; the
                        # normalizing broadcast-multiply is deferred into
                        # the next qb's instruction stream
                        dnr = dsm.tile([128, 512], f32, tag="dnr")
                        dnrb = dsm.tile([128, 512], bf16, tag="dnrb")
                        qsl = slice(qb * 512, (qb + 1) * 512)
                        # full-tile ops: reciprocal_approx_fast (a custom
                        # DVE ISA op) only works at partition base 0 on HW;
                        # rows other than 0/32/64/96 are garbage and unread
                        nh97 = 32 * (nq - 1) + 1
                        nc.vector.reciprocal_approx_fast(
                            dnr[0:nh97, :], dnB[0:nh97, :]
                        )
                        nc.vector.tensor_copy(dnrb[0:nh97, :], dnr[0:nh97, :])
                        for h in range(nq):
                            # DVE, not scalar: the scalar engine (exp) is
                            # the critical path through attention
                            nc.vector.tensor_copy(at[h][:, qsl], accs[h])

                        def _norm(alloc_bc, dnrb=dnrb, qsl=qsl):
                            for h in range(nq):
                                r = slice(32 * h, 32 * h + 1)
                                bc = alloc_bc(h)
                                nc.tensor.matmul(
                                    bc, ones32[r, :], dnrb[r, :],
                                    start=True, stop=True,
                                    tile_position=(32 * h, 0),
                                )
                                nc.vector.tensor_mul(
                                    at[h][:, qsl], at[h][:, qsl], bc
                                )

                        pending_norm = _norm

                # ---------------- Phase 3: output projection ----------------
                with (
                    tc.tile_pool(name="outp", bufs=2, space="PSUM") as outp,
                    tc.tile_pool(name="osb", bufs=3) as osb,
                ):
                    for st in range(STn):
                        ssl = slice(st * 128, (st + 1) * 128)
                        for g in range(2):
                            pos = [
                                outp.tile(
                                    [128, 512], f32, tag=f"po{b}", name=f"po{b}"
                                )
                                for b in range(4)
                            ]
                            for h in range(nq):
                                for b in range(4):
                                    nb = g * 4 + b
                                    nsl = slice(nb * 512, (nb + 1) * 512)
                                    nc.tensor.matmul(
                                        pos[b],
                                        at[h][:, ssl],
                                        wo_sb[:, h, nsl],
                                        start=(h == 0),
                                        stop=(h == nq - 1),
                                    )
                            ot = osb.tile([128, 2048], bf16, tag="ot")
                            last = st == STn - 1 and g == 1
                            for b in range(4):
                                csl = slice(b * 512, (b + 1) * 512)
                                if b % 2 == 0:
                                    nc.vector.tensor_copy(ot[:, csl], pos[b])
                                else:
                                    nc.scalar.copy(ot[:, csl], pos[b])
                                if last:
                                    # final group: per-block DMA so the
                                    # last transfer overlaps the copies
                                    # instead of serializing at the drain
                                    nc.sync.dma_start(
                                        out_d[
                                            ssl,
                                            g * 2048 + b * 512 :
                                            g * 2048 + (b + 1) * 512,
                                        ],
                                        ot[:, csl],
                                    )
                            if not last:
                                nc.sync.dma_start(
                                    out_d[ssl, g * 2048 : (g + 1) * 2048], ot
                                )
                            if pending_norm is not None:
                                # last attention block's normalization,
                                # overlapped with the first o_proj group
                                # (it only touches at[:, 1536:2048], read
                                # by st >= 12); bc tiles borrow one spare
                                # buffer from each po tag so the four
                                # broadcasts don't serialize
                                pending_norm(
                                    lambda h: outp.tile(
                                        [128, 512], f32, tag=f"po{h}",
                                        name=f"bc{h}",
                                    )
                                )
                                pending_norm = None

    nc.compile()
    nc.finalize()
    return nc


def _emit_dn_acc(nc, pair, oh4, vt, dnB, accs, qb, nkt):
    """Denominator + PV matmuls for one head-pair (one pipeline stage)."""
    kt, jstart, w, exs = pair
    nh = len(accs)
    for h, ex in exs:
        # the group-opening matmul must cover every row later matmuls
        # touch (rows 0..32*(nh-1)); extra one-hot columns are zero so
        # the wider write only adds zeros
        wide = 32 * (nh - 1) + 1 if (kt == 0 and h == 0) else 32 * h + 1
        nc.tensor.matmul(
            dnB[0:wide, jstart:512],
            oh4[:, h, 0:wide],
            ex[:, :w],
            start=(kt == 0 and h == 0),
            stop=(kt == nkt - 1 and h == nh - 1),
        )
    for h, ex in exs:
        nc.tensor.matmul(
            accs[h][:, jstart:512],
            vt[:, kt, :],
            ex[:, :w],
            start=(kt == 0),
            stop=(kt == nkt - 1),
        )


def _prep_core_inputs(x_np, position_ids, Wq, Wk, Wv, Wo):
    """Host-side sharding/layout prep. Returns list of per-core input dicts."""
    scale = float(D) ** -0.5
    xT = np.ascontiguousarray(x_np.T).astype(BF)

    pos = np.asarray(position_ids).astype(np.float32)
    half = D // 2
    inv_freq = 1.0 / (ROPE_THETA ** (np.arange(half, dtype=np.float32) / half))
    ang = pos[:, None] * inv_freq[None, :]  # [S, 64]
    cosT = np.cos(ang).T.astype(np.float32)  # [64, S]
    sinT = np.sin(ang).T.astype(np.float32)
    cos2 = np.ascontiguousarray(np.concatenate([cosT, cosT], axis=0)).astype(BF)
    sin2 = np.ascontiguousarray(np.concatenate([-sinT, sinT], axis=0)).astype(BF)

    tri = np.triu(np.ones((128, 128), np.float32)).astype(BF)  # [k, q]: q >= k
    swpm = np.zeros((128, 128), np.float32)
    swpm[np.arange(128), (np.arange(128) + 64) % 128] = 1.0  # half-swap perm
    swpm = swpm.astype(BF)

    Wq_s = (np.asarray(Wq, np.float32) * scale)
    Wk = np.asarray(Wk, np.float32)
    Wv = np.asarray(Wv, np.float32)
    Wo = np.asarray(Wo, np.float32)

    in_maps = []
    for c in range(NCORES):
        qrows = Wq_s[c * NQ * D : (c + 1) * NQ * D]  # [512, HID]
        krows = Wk[c * D : (c + 1) * D]  # [128, HID]
        vrows = Wv[c * D : (c + 1) * D]
        # column order in wqkvT: k, v, q0..q3
        wqkv = np.concatenate([krows, vrows, qrows], axis=0)  # [768, HID]
        wqkvT = np.ascontiguousarray(wqkv.T).astype(BF)  # [HID, 768]
        woT = np.ascontiguousarray(Wo[:, c * NQ * D : (c + 1) * NQ * D].T).astype(
            BF
        )  # [512, HID]
        in_maps.append(
            {
                "xT": xT,
                "wqkvT": wqkvT,
                "woT": woT,
                "cos2": cos2,
                "sin2": sin2,
                "tri": tri,
                "swpm": swpm,
            }
        )
    return in_maps


def kernel(
    hidden_states,
    position_ids,
    page_indices,
    Wq,
    Wk,
    Wv,
    Wo,
    kv_cache,
    _trace=False,
):
    from concourse.bass_utils import run_bass_kernel_spmd

    x = np.asarray(hidden_states, np.float32)[0]  # [S, HID]
    pidx = np.asarray(page_indices)
    # write-then-gather through distinct pages is the identity
    assert len(np.unique(pidx)) == pidx.shape[0], "page_indices must be distinct"

    in_maps = _prep_core_inputs(x, position_ids, Wq, Wk, Wv, Wo)

    if "nc" not in _NC_CACHE:
        _NC_CACHE["nc"] = build_nc()
    nc = _NC_CACHE["nc"]

    res = run_bass_kernel_spmd(
        nc, in_maps, core_ids=list(range(NCORES)), trace=_trace
    )
    out = np.zeros((S, HID), np.float32)
    for c in range(NCORES):
        out += np.asarray(res.results[c]["out"], np.float32)
    if _trace:
        kernel.last_results = res
    return out[None].astype(np.float32)


# revision 39
# speedup vs baseline: 1.0079x; 1.0079x over previous
"""Trainium2 Bass kernel for Llama-style GQA attention prefill (S=2048).

Sharding: tensor-parallel over heads across 8 NeuronCores.
Each core owns 4 query heads + 1 KV head (GQA group-aligned), computes
its partial o_proj contribution (Wo input-sharded), and the host sums
the 8 partials.

Math notes:
- The paged-KV write+gather in the reference is an identity whenever
  page_indices are distinct (they are: arange(128)), so the kernel
  computes plain causal GQA attention with RoPE.
- Matmuls run in bf16 (4x faster than fp32 on the PE) with fp32 PSUM
  accumulation. Score scale 1/sqrt(D) is folded into Wq on the host.
- Attention uses a transposed-score layout: scoresT[k, q] so softmax
  needs no PE transposes. exp() is taken without max subtraction
  (scores are O(10), safe in fp32).

Performance structure (per core):
- Phase 1 (projections): W stationary / x moving, N=512 tiles; RoPE's
  half-swap done with a bf16 PE permutation matmul (4x faster than the
  fp32 variant; DVE stream_shuffle cannot cross 32-partition quadrants);
  w/x DMAs interleaved so the first matmul starts ~1us in.
- Phase 2 (attention): kt-major over head pairs with a one-pair
  software pipeline so the PE never waits on the scalar-engine exp;
  the 4 heads' softmax denominators accumulate into rows 0/32/64/96 of
  a single PSUM bank (shifted one-hot stationary operands), inverted
  with reciprocal_approx_fast and broadcast via a bf16 rank-1 matmul.
  All of this stays off the PE critical path, keeping the HAM clock
  gate warm (the previous version lost ~80us to K=4/8 throttling).
- Phase 3 (o_proj): at[h] stationary shared across 4 hidden blocks
  (psum-bank-grouped), bf16 output (halves the output DMA).
"""

import sys

if "/opt/trn_rl_repo" not in sys.path:
    sys.path.insert(0, "/opt/trn_rl_repo")

import numpy as np
import ml_dtypes

BF = ml_dtypes.bfloat16

S = 2048
HID = 4096
D = 128
H = 32
HKV = 8
NCORES = 8
NQ = H // NCORES  # 4 query heads per core
ROPE_THETA = 10000.0

_NC_CACHE = {}


def build_nc(s=S, hid=HID, nq=NQ):
    """Build the per-core Bass program (same program for all 8 cores)."""
    import concourse.bass as bass
    import concourse.mybir as mybir
    import concourse.tile as tile
    from concourse import bacc
    from concourse.masks import make_identity

    f32 = mybir.dt.float32
    bf16 = mybir.dt.bfloat16
    Exp = mybir.ActivationFunctionType.Exp

    KB = hid // 128   # hidden contraction blocks
    SBn = s // 512    # 512-wide sequence blocks
    KTn = s // 128    # 128-wide key tiles
    STn = s // 128    # 128-wide seq tiles (same count)
    NDB = nq + 2      # projection d-blocks: k, v, q0..q{nq-1}
    WC = NDB * 128    # wqkvT columns
    HB = hid // 512   # output hidden blocks

    nc = bacc.Bacc("TRN2")

    xT_d = nc.dram_tensor("xT", [hid, s], bf16, kind="ExternalInput")
    w_d = nc.dram_tensor("wqkvT", [hid, WC], bf16, kind="ExternalInput")
    wo_d = nc.dram_tensor("woT", [nq * 128, hid], bf16, kind="ExternalInput")
    cos2_d = nc.dram_tensor("cos2", [128, s], bf16, kind="ExternalInput")
    sin2_d = nc.dram_tensor("sin2", [128, s], bf16, kind="ExternalInput")
    tri_d = nc.dram_tensor("tri", [128, 128], bf16, kind="ExternalInput")
    swpm_d = nc.dram_tensor("swpm", [128, 128], bf16, kind="ExternalInput")
    out_d = nc.dram_tensor("out", [s, hid], bf16, kind="ExternalOutput")

    with tile.TileContext(nc) as tc:
        with (
            tc.tile_pool(name="const", bufs=1) as const_pool,
            tc.tile_pool(name="qkv", bufs=1) as qkv_pool,
        ):
            tri = const_pool.tile([128, 128], bf16, tag="tri")
            # per-head one-hot slabs: oh4[:, h, 32h] = 1, zero elsewhere
            oh4 = const_pool.tile([128, nq, 128], bf16, tag="oh4")
            ones32 = const_pool.tile([128, 128], bf16, tag="ones32")
            ident = const_pool.tile([128, 128], bf16, tag="ident")
            swpm = const_pool.tile([128, 128], bf16, tag="swpm")
            nc.sync.dma_start(tri, tri_d[:, :])
            nc.sync.dma_start(swpm, swpm_d[:, :])
            nc.vector.memset(oh4, 0.0)
            for h in range(nq):
                nc.vector.memset(oh4[:, h, 32 * h : 32 * h + 1], 1.0)
            nc.vector.memset(ones32, 1.0)
            make_identity(nc, ident)

            # persistent per-head tensors
            qk = [
                qkv_pool.tile([128, s], bf16, tag=f"qk{i}", name=f"qk{i}")
                for i in range(nq + 1)
            ]  # qk[0..nq-1] = q heads (T layout [d, s]); qk[nq] = kT
            vt = qkv_pool.tile([128, KTn, 128], bf16, tag="vt")  # v natural [s,d] tiles

            # ---------------- Phase 1: projections + RoPE ----------------
            with (
                tc.tile_pool(name="cs", bufs=1) as cs_pool,
                tc.tile_pool(name="xt", bufs=2) as xt_pool,
                tc.tile_pool(name="wsb", bufs=1) as w_pool,
                tc.tile_pool(name="pp", bufs=4, space="PSUM") as pp,
                tc.tile_pool(name="tpp", bufs=2, space="PSUM") as tpp,
                tc.tile_pool(name="spp", bufs=2, space="PSUM") as spp,
                tc.tile_pool(name="rtmp", bufs=2) as rt,
                tc.tile_pool(name="vstage", bufs=2) as vs,
            ):
                cos2 = cs_pool.tile([128, s], bf16, tag="cos2")
                sin2 = cs_pool.tile([128, s], bf16, tag="sin2")
                w_sb = w_pool.tile([128, KB, WC], bf16, tag="wsb")
                wv_view = w_d[:, :].rearrange("(t p) c -> p t c", p=128)

                # interleave w and first-seq-block x DMAs so the first
                # matmul can start after ~2 transfers instead of ~6 MB
                xts_all = []
                for sb in range(SBn):
                    xts_all.append(
                        [
                            xt_pool.tile(
                                [128, 512], bf16, tag=f"xt{kb}", name=f"xt{kb}"
                            )
                            for kb in range(KB)
                        ]
                    )
                for kb in range(KB):
                    nc.sync.dma_start(w_sb[:, kb, :], wv_view[:, kb, :])
                    nc.sync.dma_start(
                        xts_all[0][kb], xT_d[kb * 128 : (kb + 1) * 128, 0:512]
                    )
                    if kb == 24:
                        # cos/sin aren't needed until the first RoPE block
                        # (~40us in); issuing them here keeps the first
                        # matmuls' w/x transfers at the head of the queue
                        nc.sync.dma_start(cos2, cos2_d[:, :])
                        nc.sync.dma_start(sin2, sin2_d[:, :])

                # d-block order: k(0), v(1), then q heads (2..)
                for sb in range(SBn):
                    sl = slice(sb * 512, (sb + 1) * 512)
                    xts = xts_all[sb]
                    if sb > 0:
                        for kb in range(KB):
                            nc.sync.dma_start(
                                xts[kb], xT_d[kb * 128 : (kb + 1) * 128, sl]
                            )
                    for db in range(NDB):
                        ps = pp.tile([128, 512], f32, tag="pp")
                        for kb in range(KB):
                            nc.tensor.matmul(
                                ps,
                                w_sb[:, kb, db * 128 : (db + 1) * 128],
                                xts[kb],
                                start=(kb == 0),
                                stop=(kb == KB - 1),
                            )
                        if db == 1:
                            # v: cast to bf16 then transpose to natural [s, d]
                            vstg = vs.tile([128, 512], bf16, tag="vstg")
                            nc.scalar.copy(vstg, ps)
                            for j in range(4):
                                tps = tpp.tile([128, 128], bf16, tag="tpp")
                                nc.tensor.transpose(
                                    tps, vstg[:, j * 128 : (j + 1) * 128], ident
                                )
                                nc.vector.tensor_copy(vt[:, sb * 4 + j, :], tps)
                        else:
                            # RoPE: dst = p * COS2 + halfswap(p) * SIN2
                            # (half-swap via a bf16 PE permutation matmul;
                            # sin2 rows 0-63 hold -sin so one form works)
                            dst = qk[nq] if db == 0 else qk[db - 2]
                            pcs = rt.tile([128, 512], bf16, tag="pcs")
                            nc.scalar.copy(pcs, ps)
                            sps = spp.tile([128, 512], f32, tag="sps")
                            nc.tensor.matmul(sps, swpm, pcs, start=True, stop=True)
                            m1 = rt.tile([128, 512], f32, tag="m1")
                            nc.vector.tensor_mul(m1, ps, cos2[:, sl])
                            swp = rt.tile([128, 512], f32, tag="swp")
                            nc.vector.tensor_mul(swp, sps, sin2[:, sl])
                            nc.vector.tensor_add(dst[:, sl], m1, swp)

            # ---------------- Phase 2: attention ----------------
            kT = qk[nq]
            with (
                tc.tile_pool(name="wosb", bufs=1) as wo_pool,
                # dsm outlives phase 2: the last block's normalization is
                # emitted inside phase 3 (its bcast matmuls read dnrb there)
                tc.tile_pool(name="dsm", bufs=2) as dsm,
            ):
                at = [
                    wo_pool.tile([128, s], bf16, tag=f"at{i}", name=f"at{i}")
                    for i in range(nq)
                ]  # attnT per head [d, s]
                wo_sb = wo_pool.tile([128, nq, hid], bf16, tag="wosb")
                nc.sync.dma_start(
                    wo_sb, wo_d[:, :].rearrange("(t p) c -> p t c", p=128)
                )
                with (
                    # 3 score banks + 1 denominator bank (was 2+2): the
                    # extra score slack keeps the PE fed across block
                    # boundaries; the dn rotation dependency is covered
                    # by the reciprocal finishing ~1us into each block
                    tc.tile_pool(name="scp", bufs=3, space="PSUM") as scp,
                    tc.tile_pool(name="atp", bufs=1, space="PSUM") as atp,
                    tc.tile_pool(name="dnp", bufs=1, space="PSUM") as dnp,
                    tc.tile_pool(name="exps", bufs=3) as exps,
                ):
                    pending_norm = None

                    for qb in range(SBn):
                        nkt = 4 * qb + 4
                        accs = [
                            atp.tile(
                                [128, 512], f32, tag=f"acc{h}", name=f"acc{h}"
                            )
                            for h in range(nq)
                        ]
                        dnB = dnp.tile([128, 512], f32, tag="dn")

                        # software pipeline over (kt, head-pair) with one
                        # pair of lag so dn/acc matmuls never wait on exp
                        pending_pair = None
                        pair_idx = 0
                        for kt in range(nkt):
                            jstart = max(0, 128 * (kt - 4 * qb))
                            w = 512 - jstart
                            q_lo = qb * 512 + jstart
                            diag = kt >= 4 * qb
                            for hp in range(2):
                                exs = []
                                for h in (2 * hp, 2 * hp + 1):
                                    sc = scp.tile([128, 512], f32, tag="sc")
                                    nc.tensor.matmul(
                                        sc[:, :w],
                                        kT[:, kt * 128 : (kt + 1) * 128],
                                        qk[h][:, q_lo : (qb + 1) * 512],
                                        start=True,
                                        stop=True,
                                    )
                                    ex = exps.tile(
                                        [128, 512], bf16, tag=f"ex{h}",
                                        name=f"ex{h}",
                                    )
                                    nc.scalar.activation(
                                        ex[:, :w], sc[:, :w], Exp
                                    )
                                    if diag:
                                        nc.vector.tensor_mul(
                                            ex[:, 0:128], ex[:, 0:128], tri
                                        )
                                    exs.append((h, ex))
                                if pending_pair is not None:
                                    _emit_dn_acc(
                                        nc, pending_pair, oh4, vt, dnB, accs,
                                        qb, nkt,
                                    )
                                pending_pair = (kt, jstart, w, exs)
                                pair_idx += 1
                                if pair_idx == 2 and pending_norm is not None:
                                    # previous block's normalizing broadcast:
                                    # two pairs in, the reciprocal chain has
                                    # certainly finished, so the PE won't wait
                                    pending_norm(
                                        lambda h: scp.tile(
                                            [128, 512], f32, tag="sc",
                                            name="bc",
                                        )
                                    )
                                    pending_norm = None
                        _emit_dn_acc(
                            nc, pending_pair, oh4, vt, dnB, accs, qb, nkt
                        )

                        # epilogue: invert denominators (rows 0/32/64/96 of
                        # dnB) and copy unnormalized acc into at[h]; the
                        # normalizing broadcast-multiply is deferred into
                        # the next qb's instruction stream
                        dnr = dsm.tile([128, 512], f32, tag="dnr")
                        dnrb = dsm.tile([128, 512], bf16, tag="dnrb")
                        qsl = slice(qb * 512, (qb + 1) * 512)
                        # full-tile ops: reciprocal_approx_fast (a custom
                        # DVE ISA op) only works at partition base 0 on HW;
                        # rows other than 0/32/64/96 are garbage and unread
                        nh97 = 32 * (nq - 1) + 1
                        nc.vector.reciprocal_approx_fast(
                            dnr[0:nh97, :], dnB[0:nh97, :]
                        )
                        nc.vector.tensor_copy(dnrb[0:nh97, :], dnr[0:nh97, :])
                        for h in range(nq):
                            # DVE, not scalar: the scalar engine (exp) is
                            # the critical path through attention
                            nc.vector.tensor_copy(at[h][:, qsl], accs[h])

                        def _norm(alloc_bc, dnrb=dnrb, qsl=qsl):
                            for h in range(nq):
                                r = slice(32 * h, 32 * h + 1)
                                bc = alloc_bc(h)
                                nc.tensor.matmul(
                                    bc, ones32[r, :], dnrb[r, :],
                                    start=True, stop=True,
                                    tile_position=(32 * h, 0),
                                )
                                nc.vector.tensor_mul(
                                    at[h][:, qsl], at[h][:, qsl], bc
                                )

                        pending_norm = _norm

                # ---------------- Phase 3: output projection ----------------
                with (
                    tc.tile_pool(name="outp", bufs=2, space="PSUM") as outp,
                    tc.tile_pool(name="osb", bufs=3) as osb,
                ):
                    for st in range(STn):
                        ssl = slice(st * 128, (st + 1) * 128)
                        for g in range(2):
                            pos = [
                                outp.tile(
                                    [128, 512], f32, tag=f"po{b}", name=f"po{b}"
                                )
                                for b in range(4)
                            ]
                            for h in range(nq):
                                for b in range(4):
                                    nb = g * 4 + b
                                    nsl = slice(nb * 512, (nb + 1) * 512)
                                    nc.tensor.matmul(
                                        pos[b],
                                        at[h][:, ssl],
                                        wo_sb[:, h, nsl],
                                        start=(h == 0),
                                        stop=(h == nq - 1),
                                    )
                            ot = osb.tile([128, 2048], bf16, tag="ot")
                            for b in range(4):
                                csl = slice(b * 512, (b + 1) * 512)
                                if b % 2 == 0:
                                    nc.vector.tensor_copy(ot[:, csl], pos[b])
                                else:
                                    nc.scalar.copy(ot[:, csl], pos[b])
                            nc.sync.dma_start(
                                out_d[ssl, g * 2048 : (g + 1) * 2048], ot
                            )
                            if pending_norm is not None:
                                # last attention block's normalization,
                                # overlapped with the first o_proj group
                                # (it only touches at[:, 1536:2048], read
                                # by st >= 12); bc tiles borrow one spare
                                # buffer from each po tag so the four
                                # broadcasts don't serialize
                                pending_norm(
                                    lambda h: outp.tile(
                                        [128, 512], f32, tag=f"po{h}",
                                        name=f"bc{h}",
                                    )
                                )
                                pending_norm = None

    nc.compile()
    nc.finalize()
    return nc


def _emit_dn_acc(nc, pair, oh4, vt, dnB, accs, qb, nkt):
    """Denominator + PV matmuls for one head-pair (one pipeline stage)."""
    kt, jstart, w, exs = pair
    nh = len(accs)
    for h, ex in exs:
        # the group-opening matmul must cover every row later matmuls
        # touch (rows 0..32*(nh-1)); extra one-hot columns are zero so
        # the wider write only adds zeros
        wide = 32 * (nh - 1) + 1 if (kt == 0 and h == 0) else 32 * h + 1
        nc.tensor.matmul(
            dnB[0:wide, jstart:512],
            oh4[:, h, 0:wide],
            ex[:, :w],
            start=(kt == 0 and h == 0),
            stop=(kt == nkt - 1 and h == nh - 1),
        )
    for h, ex in exs:
        nc.tensor.matmul(
            accs[h][:, jstart:512],
            vt[:, kt, :],
            ex[:, :w],
            start=(kt == 0),
            stop=(kt == nkt - 1),
        )


def _prep_core_inputs(x_np, position_ids, Wq, Wk, Wv, Wo):
    """Host-side sharding/layout prep. Returns list of per-core input dicts."""
    scale = float(D) ** -0.5
    xT = np.ascontiguousarray(x_np.T).astype(BF)

    pos = np.asarray(position_ids).astype(np.float32)
    half = D // 2
    inv_freq = 1.0 / (ROPE_THETA ** (np.arange(half, dtype=np.float32) / half))
    ang = pos[:, None] * inv_freq[None, :]  # [S, 64]
    cosT = np.cos(ang).T.astype(np.float32)  # [64, S]
    sinT = np.sin(ang).T.astype(np.float32)
    cos2 = np.ascontiguousarray(np.concatenate([cosT, cosT], axis=0)).astype(BF)
    sin2 = np.ascontiguousarray(np.concatenate([-sinT, sinT], axis=0)).astype(BF)

    tri = np.triu(np.ones((128, 128), np.float32)).astype(BF)  # [k, q]: q >= k
    swpm = np.zeros((128, 128), np.float32)
    swpm[np.arange(128), (np.arange(128) + 64) % 128] = 1.0  # half-swap perm
    swpm = swpm.astype(BF)

    Wq_s = (np.asarray(Wq, np.float32) * scale)
    Wk = np.asarray(Wk, np.float32)
    Wv = np.asarray(Wv, np.float32)
    Wo = np.asarray(Wo, np.float32)

    in_maps = []
    for c in range(NCORES):
        qrows = Wq_s[c * NQ * D : (c + 1) * NQ * D]  # [512, HID]
        krows = Wk[c * D : (c + 1) * D]  # [128, HID]
        vrows = Wv[c * D : (c + 1) * D]
        # column order in wqkvT: k, v, q0..q3
        wqkv = np.concatenate([krows, vrows, qrows], axis=0)  # [768, HID]
        wqkvT = np.ascontiguousarray(wqkv.T).astype(BF)  # [HID, 768]
        woT = np.ascontiguousarray(Wo[:, c * NQ * D : (c + 1) * NQ * D].T).astype(
            BF
        )  # [512, HID]
        in_maps.append(
            {
                "xT": xT,
                "wqkvT": wqkvT,
                "woT": woT,
                "cos2": cos2,
                "sin2": sin2,
                "tri": tri,
                "swpm": swpm,
            }
        )
    return in_maps


def kernel(
    hidden_states,
    position_ids,
    page_indices,
    Wq,
    Wk,
    Wv,
    Wo,
    kv_cache,
    _trace=False,
):
    from concourse.bass_utils import run_bass_kernel_spmd

    x = np.asarray(hidden_states, np.float32)[0]  # [S, HID]
    pidx = np.asarray(page_indices)
    # write-then-gather through distinct pages is the identity
    assert len(np.unique(pidx)) == pidx.shape[0], "page_indices must be distinct"

    in_maps = _prep_core_inputs(x, position_ids, Wq, Wk, Wv, Wo)

    if "nc" not in _NC_CACHE:
        _NC_CACHE["nc"] = build_nc()
    nc = _NC_CACHE["nc"]

    res = run_bass_kernel_spmd(
        nc, in_maps, core_ids=list(range(NCORES)), trace=_trace
    )
    out = np.zeros((S, HID), np.float32)
    for c in range(NCORES):
        out += np.asarray(res.results[c]["out"], np.float32)
    if _trace:
        kernel.last_results = res
    return out[None].astype(np.float32)
